# revision 2
# baseline (speedup 1.0000x reference)
"""Fused multi-head attention on 8 Trainium2 NeuronCores — v2.

vs v1 baseline:
  - q/k projection in fp8 (e4m3) DoubleRow mode: 4-step chains at 2x rate.
    Weights pre-scaled by 16 on host (into e4m3 normal range); 1/16 folded
    into the bias tensor_scalar.
  - Attention output o is pre-normalized (divide by softmax colsum) BEFORE
    the out-projection, so out-proj is a single K=128 matmul over both
    heads and the huge per-tile DVE rescale epilogue disappears.
    colsum rides the PV matmul as row 64 ([v|ones] aug layout); recip on
    DVE (bf16), broadcast across partitions via a tiny K=1 ones-matmul on
    the PE, normalized with one tensor_tensor per head half.
  - Output written bf16 (host sums partials in f32).

Layouts per core (2 heads A=2c, B=2c+1):
  xT    [128,4096]x8  bf16   hidden on partitions, tokens free (b-major)
  xf8   [128,2,4096]x4 fp8   DoubleRow pairs of hidden k-subtiles
  qkvT  [128,4096]x2  bf16   q/k feature-major (A dims 0-63, B 64-127)
  vaug  [128,16,2,65] bf16   per kt, per head: [v(64) | ones(1)]
  o_ps  [65,512]x2    PSUM   rows 0-63 = o, row 64 = colsum
  oT    [128,512]     bf16   normalized, A on parts 0-63, B on 64-127
  y     [128,512]     PSUM   K=128 out-proj; copied to bf16, DMA'd out
"""

import sys
import types
import numpy as np
import ml_dtypes

import concourse.bass as bass
import concourse.tile as tile
from concourse import bacc, mybir

BF16 = mybir.dt.bfloat16
F32 = mybir.dt.float32
FP8 = mybir.dt.float8e4
BF16_NP = ml_dtypes.bfloat16
FP8_NP = ml_dtypes.float8_e4m3

B, S, H, NH, HD = 2, 2048, 1024, 16, 64
T = B * S               # 4096 tokens, b-major
NCORES = 8
HPC = NH // NCORES      # heads per core = 2
DPC = HPC * HD          # head dims per core = 128
KT = 128                # keys per k-tile
NKT = S // KT           # 16
QC = 512                # query chunk
NQC = S // QC           # 4
HKT = H // 128          # hidden k-tiles = 8
NDR = HKT // 2          # DoubleRow chain steps = 4
WSCALE = 16.0           # host pre-scale on q/k weights
EXPSCALE = 1.0 / np.sqrt(HD)

_CACHED = {}


def _build_nc():
    nc = bacc.Bacc(None, target_bir_lowering=False, debug=False)
    xT = nc.dram_tensor("xT", [H, T], BF16, kind="ExternalInput").ap()
    xf8 = nc.dram_tensor("xf8", [NDR, 128, 2, T], FP8, kind="ExternalInput").ap()
    wqk8 = nc.dram_tensor("wqk8", [NDR, 128, 2, 2 * 128], FP8,
                          kind="ExternalInput").ap()
    wv = nc.dram_tensor("wv", [HKT, 128, DPC], BF16, kind="ExternalInput").ap()
    woT = nc.dram_tensor("woT", [DPC, H], BF16, kind="ExternalInput").ap()
    bqk = nc.dram_tensor("bqk", [128, 2], F32, kind="ExternalInput").ap()
    vbias = nc.dram_tensor("vbias", [128, DPC], F32, kind="ExternalInput").ap()
    out = nc.dram_tensor("out", [T, H], BF16, kind="ExternalOutput").ap()
    import os
    dbg = os.environ.get("K2DEBUG") == "1"
    if dbg:
        dvaug = nc.dram_tensor("dvaug", [128, NKT, HPC, HD + 1], BF16,
                               kind="ExternalOutput").ap()
        dvt = nc.dram_tensor("dvt", [128, T], BF16, kind="ExternalOutput").ap()
        dqk = nc.dram_tensor("dqk", [128, T], BF16, kind="ExternalOutput").ap()

    EXP = mybir.ActivationFunctionType.Exp
    MULT = mybir.AluOpType.mult
    ADD = mybir.AluOpType.add
    DR = mybir.MatmulPerfMode.DoubleRow

    with tile.TileContext(nc) as tc:
        with (
            tc.tile_pool(name="const", bufs=1) as constp,
            tc.tile_pool(name="xw", bufs=1) as xwp,
            tc.tile_pool(name="qkv", bufs=1) as qkvp,
            tc.tile_pool(name="vaug", bufs=1) as vaugp,
            tc.tile_pool(name="oT", bufs=2) as oTp,
            tc.tile_pool(name="p", bufs=3) as pp,
            tc.tile_pool(name="ysb", bufs=3) as ysbp,
            tc.tile_pool(name="recip", bufs=2) as recipp,
            tc.tile_pool(name="rep", bufs=2) as repp,
            tc.tile_pool(name="ps", bufs=2, space="PSUM") as psp,
            tc.tile_pool(name="pso", bufs=2, space="PSUM") as psop,
        ):
            # ---- weights + x in; b0 halves first (v4 schedule) ----
            bias_sb = constp.tile([128, 2], F32, tag="bias")
            nc.sync.dma_start(bias_sb[:], bqk[:])
            vbias_sb = constp.tile([128, DPC], F32, tag="vbias")
            nc.sync.dma_start(vbias_sb[:], vbias[:])
            wo_sb = constp.tile([DPC, H], BF16, tag="wo")
            nc.sync.dma_start(wo_sb[:], woT[:])
            ones_sb = constp.tile([128, 128], BF16, tag="ones")
            nc.vector.memset(ones_sb[:], 1.0)

            wqk_sb = [constp.tile([128, 2, 256], FP8, name=f"wqk{s}", tag=f"wqk{s}")
                      for s in range(NDR)]
            for s in range(NDR):
                nc.sync.dma_start(wqk_sb[s][:], wqk8[s])
            wv_sb = [constp.tile([128, DPC], BF16, name=f"wv{k}", tag=f"wv{k}")
                     for k in range(HKT)]
            for k in range(HKT):
                nc.sync.dma_start(wv_sb[k][:], wv[k])

            xf8_sb = [xwp.tile([128, 2, T], FP8, name=f"xf8{s}", tag=f"xf8{s}")
                      for s in range(NDR)]
            xT_sb = [xwp.tile([128, T], BF16, name=f"xsb{k}", tag=f"x{k}")
                     for k in range(HKT)]
            for half in range(2):
                sl = slice(half * S, (half + 1) * S)
                for s in range(NDR):
                    eng = nc.sync if s % 2 == 0 else nc.scalar
                    eng.dma_start(xf8_sb[s][:, :, sl], xf8[s][:, :, sl])
                for k in range(HKT):
                    eng = nc.sync if k % 2 == 0 else nc.scalar
                    eng.dma_start(xT_sb[k][:, sl], xT[k * 128:(k + 1) * 128, sl])

            # vaug tiles (memset to 1.0 so the ones columns are ready)
            vaug = {}
            for b in range(B):
                va = vaugp.tile([128, NKT, HPC, HD + 1], BF16,
                                name=f"va{b}", tag=f"va{b}")
                nc.vector.memset(va[:], 1.0)
                vaug[b] = va

            qkvT_sb = {
                fg: qkvp.tile([128, T], BF16, name=f"qkvsb{fg}", tag=f"qkv{fg}")
                for fg in range(2)
            }

            # ---- projection steps ----
            def v_tile_step(b, kt):
                def run():
                    v_ps = psp.tile([128, DPC], F32, name=f"vps{b}{kt}", tag="y")
                    for k in range(HKT):
                        nc.tensor.matmul(
                            v_ps[:],
                            lhsT=xT_sb[k][:, b * S + kt * KT:b * S + (kt + 1) * KT],
                            rhs=wv_sb[k][:],
                            start=(k == 0), stop=(k == HKT - 1),
                        )
                    nc.vector.tensor_add(
                        vaug[b][:, kt, :, 0:HD],
                        v_ps[:].rearrange("p (j d) -> p j d", j=HPC),
                        vbias_sb[:].rearrange("p (j d) -> p j d", j=HPC),
                    )
                return run

            def qk_chunk_step(fg, half, t):
                # fp8 DoubleRow chain: 4 steps of 256-deep contraction
                def run():
                    ps = psp.tile([128, 512], F32, name=f"qkc{fg}{half}{t}", tag="y")
                    for s in range(NDR):
                        nc.tensor.matmul(
                            ps[:],
                            lhsT=wqk_sb[s][:, :, fg * 128:(fg + 1) * 128],
                            rhs=xf8_sb[s][:, :,
                                          half * S + t * 512:half * S + (t + 1) * 512],
                            start=(s == 0), stop=(s == NDR - 1),
                            perf_mode=DR,
                        )
                    nc.vector.tensor_scalar(
                        out=qkvT_sb[fg][:, half * S + t * 512:half * S + (t + 1) * 512],
                        in0=ps[:],
                        scalar1=1.0 / WSCALE,
                        scalar2=bias_sb[:, fg:fg + 1],
                        op0=MULT, op1=ADD,
                    )
                return run

            # ---- b0 projections up front ----
            for fg in (0, 1):
                for t in range(4):
                    qk_chunk_step(fg, 0, t)()

            qT_sb, kT_sb = qkvT_sb[0], qkvT_sb[1]

            # ---- attention + pipelined epilogue ----
            pending = [v_tile_step(0, kt) for kt in range(NKT)]
            extra = [v_tile_step(1, kt) for kt in range(NKT)]
            # order the b1 q/k chunks so the ones b1-qc0 scores need come first
            extra += [qk_chunk_step(fg, 1, t)
                      for fg, t in ((1, 0), (0, 0), (1, 1), (1, 2), (1, 3),
                                    (0, 1), (0, 2), (0, 3))]

            def make_epilogue(b, qc, oT_raw, csrow):
                q0 = b * S + qc * QC
                state = {}

                def bcast_step():
                    # broadcast the bf16 colsum row across partitions via
                    # K=1 ones-matmuls, then 1/x on the [128,512] tiles
                    # (reciprocal_approx_fast doubles as the PSUM->SBUF copy;
                    # it is broken on 1-partition APs, fine on 128)
                    csA_ps = psp.tile([128, 512], F32, name=f"rA{b}{qc}", tag="y")
                    csB_ps = psp.tile([128, 512], F32, name=f"rB{b}{qc}", tag="y")
                    nc.tensor.matmul(
                        csA_ps[:], lhsT=ones_sb[64:65, :],
                        rhs=csrow[64:65, 0:512], start=True, stop=True,
                    )
                    nc.tensor.matmul(
                        csB_ps[:], lhsT=ones_sb[64:65, :],
                        rhs=csrow[64:65, 512:1024], start=True, stop=True,
                    )
                    repA = repp.tile([128, 512], F32, name=f"rsA{b}{qc}", tag="repA")
                    repB = repp.tile([128, 512], F32, name=f"rsB{b}{qc}", tag="repB")
                    nc.vector.reciprocal_approx_fast(repA[:], csA_ps[:])
                    nc.vector.reciprocal_approx_fast(repB[:], csB_ps[:])
                    state["repA"], state["repB"] = repA, repB

                def norm_step():
                    oT = oTp.tile([128, QC], BF16, name=f"oT{b}{qc}", tag="oT")
                    nc.vector.tensor_tensor(
                        oT[0:64, :], oT_raw[0:64, :], state["repA"][0:64, :],
                        op=MULT,
                    )
                    nc.vector.tensor_tensor(
                        oT[64:128, :], oT_raw[64:128, :], state["repB"][64:128, :],
                        op=MULT,
                    )
                    state["oT"] = oT

                def y_step(tt, ec):
                    def run():
                        y_ps = psp.tile([128, 512], F32,
                                        name=f"y{b}{qc}{tt}{ec}", tag="y")
                        nc.tensor.matmul(
                            y_ps[:],
                            lhsT=state["oT"][:, tt * KT:(tt + 1) * KT],
                            rhs=wo_sb[:, ec * 512:(ec + 1) * 512],
                            start=True, stop=True,
                        )
                        y_sb = ysbp.tile([128, 512], BF16,
                                         name=f"ys{b}{qc}{tt}{ec}", tag="ysb")
                        nc.vector.tensor_copy(y_sb[:], y_ps[:])
                        nc.gpsimd.dma_start(
                            out[q0 + tt * KT:q0 + (tt + 1) * KT,
                                ec * 512:(ec + 1) * 512],
                            y_sb[:],
                        )
                    return run

                steps = [bcast_step, norm_step]
                for tt in range(4):
                    for ec in range(2):
                        steps.append(y_step(tt, ec))
                return steps

            for b in range(B):
                for qc in range(NQC):
                    q0 = b * S + qc * QC
                    o_ps = [psop.tile([HD + 1, QC], F32,
                                      name=f"o{b}{qc}{h}", tag="o")
                            for h in range(HPC)]
                    p_tiles = []
                    for kt in range(NKT):
                        s_ps = psp.tile([128, HPC * QC], F32, tag="s")
                        for h in range(HPC):
                            nc.tensor.matmul(
                                s_ps[:, h * QC:(h + 1) * QC],
                                lhsT=kT_sb[h * HD:(h + 1) * HD,
                                           b * S + kt * KT:b * S + (kt + 1) * KT],
                                rhs=qT_sb[h * HD:(h + 1) * HD, q0:q0 + QC],
                                start=True, stop=True,
                            )
                        p_sb = pp.tile([128, HPC * QC], BF16, tag="p")
                        nc.scalar.activation(p_sb[:], s_ps[:], EXP,
                                             scale=float(EXPSCALE))
                        p_tiles.append(p_sb)
                        if pending:
                            pending.pop(0)()
                        if pending and len(pending) > NKT - 1 - kt:
                            pending.pop(0)()
                        if kt >= 2:
                            for h in range(HPC):
                                nc.tensor.matmul(
                                    o_ps[h][:],
                                    lhsT=vaug[b][:, kt - 2, h, :],
                                    rhs=p_tiles[kt - 2][:, h * QC:(h + 1) * QC],
                                    start=(kt - 2 == 0), stop=False,
                                )
                    for lag in (NKT - 2, NKT - 1):
                        for h in range(HPC):
                            nc.tensor.matmul(
                                o_ps[h][:],
                                lhsT=vaug[b][:, lag, h, :],
                                rhs=p_tiles[lag][:, h * QC:(h + 1) * QC],
                                start=False, stop=(lag == NKT - 1),
                            )
                    # free o_ps fast: raw o to SBUF bf16 + colsum rows
                    # (row 64) to a bf16 SBUF row for the broadcast matmul
                    oT_raw = oTp.tile([128, QC], BF16,
                                      name=f"oR{b}{qc}", tag="oTraw")
                    nc.vector.tensor_copy(oT_raw[0:64, :], o_ps[0][0:64, :])
                    nc.vector.tensor_copy(oT_raw[64:128, :], o_ps[1][0:64, :])
                    csrow = recipp.tile([128, 2 * QC], BF16,
                                        name=f"rc{b}{qc}", tag="recip")
                    nc.vector.tensor_copy(csrow[64:65, 0:512],
                                          o_ps[0][64:65, :])
                    nc.vector.tensor_copy(csrow[64:65, 512:1024],
                                          o_ps[1][64:65, :])
                    while pending:
                        pending.pop(0)()
                    epi = make_epilogue(b, qc, oT_raw, csrow)
                    # 18 > NKT: the 2-pops-per-kt drain rule clears the
                    # backlog within one chunk; ensures all of `extra` is
                    # emitted during b0 so b1 scores never lead their inputs.
                    take = min(len(extra), 18 - len(epi))
                    pending = extra[:take] + epi
                    del extra[:take]
            while pending:
                pending.pop(0)()
            if dbg:
                nc.sync.dma_start(dvaug[:], vaug[0][:])
                nc.sync.dma_start(dvt[:], vT_sb[:])
                nc.sync.dma_start(dqk[:], qkvT_sb[1][:])
    nc.compile()
    return nc


def _get_nc():
    if "nc" not in _CACHED:
        _CACHED["nc"] = _build_nc()
    return _CACHED["nc"]


def _host_prep(x, qkv_w, qkv_b, out_w):
    x = np.asarray(x, dtype=np.float32)
    qkv_w = np.asarray(qkv_w, dtype=np.float32)
    qkv_b = np.asarray(qkv_b, dtype=np.float32)
    out_w = np.asarray(out_w, dtype=np.float32)

    xTf = np.ascontiguousarray(x.reshape(T, H).T)
    xT = xTf.astype(BF16_NP)
    # fp8 x in DoubleRow layout: [NDR, 128, 2, T]
    xf8 = np.ascontiguousarray(
        xTf.reshape(NDR, 2, 128, T).transpose(0, 2, 1, 3)
    ).astype(FP8_NP)

    in_maps = []
    for c in range(NCORES):
        wq = qkv_w[128 * c:128 * c + 128]          # [128, H]
        wk = qkv_w[H + 128 * c:H + 128 * c + 128]
        wv_c = qkv_w[2 * H + 128 * c:2 * H + 128 * c + 128]
        # wqk8: [NDR, 128, 2, 256]; cols 0-127 = q feats, 128-255 = k feats
        wqk = np.concatenate([wq, wk], 0).T * WSCALE   # [H, 256]
        wqk8 = np.ascontiguousarray(
            wqk.reshape(NDR, 2, 128, 256).transpose(0, 2, 1, 3)
        ).astype(FP8_NP)
        # wv: [HKT, 128, DPC] = hidden-major slices of wv_c.T
        wvT = wv_c.T                                    # [H, DPC]
        wv8 = np.ascontiguousarray(
            wvT.reshape(HKT, 128, DPC)).astype(BF16_NP)
        woTa = np.ascontiguousarray(
            out_w[:, DPC * c:DPC * (c + 1)].T).astype(BF16_NP)  # [DPC, H]
        bqk = np.stack(
            [qkv_b[fg * H + 128 * c:fg * H + 128 * c + 128] for fg in range(2)],
            axis=1,
        ).astype(np.float32)
        vb = np.broadcast_to(
            qkv_b[2 * H + 128 * c:2 * H + 128 * c + 128][None, :], (128, DPC)
        ).astype(np.float32)
        in_maps.append({
            "xT": xT,
            "xf8": xf8,
            "wqk8": wqk8,
            "wv": wv8,
            "woT": woTa,
            "bqk": np.ascontiguousarray(bqk),
            "vbias": np.ascontiguousarray(vb),
        })
    return in_maps


def _run(in_maps, trace=False):
    if trace and "antenv.axon_hooks" not in sys.modules:
        try:
            import trn_agent_boot.trn_boot as _tb
            _hook = _tb._ntff_profile_via_ctypes("/opt/axon/libaxon_pjrt.so")
            _m = types.ModuleType("antenv.axon_hooks")
            _m.get_axon_ntff_profile_hook = lambda: _hook
            sys.modules["antenv.axon_hooks"] = _m
        except Exception:
            trace = False
    from concourse.bass_utils import run_bass_kernel_spmd

    nc = _get_nc()
    res = run_bass_kernel_spmd(nc, in_maps, core_ids=list(range(NCORES)), trace=trace)
    return res


def kernel(x, qkv_w, qkv_b, out_w, out_b):
    in_maps = _host_prep(x, qkv_w, qkv_b, out_w)
    res = _run(in_maps, trace=False)
    total = np.zeros((T, H), np.float32)
    for c in range(NCORES):
        total += res.results[c]["out"].astype(np.float32)
    total += np.asarray(out_b, dtype=np.float32)[None, :]
    return total.reshape(B, S, H)


# revision 3
# speedup vs baseline: 1.2412x; 1.2412x over previous
"""Fused multi-head attention on 8 Trainium2 NeuronCores — v2.

vs v1 baseline:
  - q/k projection in fp8 (e4m3) DoubleRow mode: 4-step chains at 2x rate.
    Weights pre-scaled by 16 on host (into e4m3 normal range); 1/16 folded
    into the bias tensor_scalar.
  - Attention output o is pre-normalized (divide by softmax colsum) BEFORE
    the out-projection, so out-proj is a single K=128 matmul over both
    heads and the huge per-tile DVE rescale epilogue disappears.
    colsum rides the PV matmul as row 64 ([v|ones] aug layout); recip on
    DVE (bf16), broadcast across partitions via a tiny K=1 ones-matmul on
    the PE, normalized with one tensor_tensor per head half.
  - Output written bf16 (host sums partials in f32).

Layouts per core (2 heads A=2c, B=2c+1):
  xT    [128,4096]x8  bf16   hidden on partitions, tokens free (b-major)
  xf8   [128,2,4096]x4 fp8   DoubleRow pairs of hidden k-subtiles
  qkvT  [128,4096]x2  bf16   q/k feature-major (A dims 0-63, B 64-127)
  vaug  [128,16,2,65] bf16   per kt, per head: [v(64) | ones(1)]
  o_ps  [65,512]x2    PSUM   rows 0-63 = o, row 64 = colsum
  oT    [128,512]     bf16   normalized, A on parts 0-63, B on 64-127
  y     [128,512]     PSUM   K=128 out-proj; copied to bf16, DMA'd out
"""

import sys
import types
import numpy as np
import ml_dtypes

import concourse.bass as bass
import concourse.tile as tile
from concourse import bacc, mybir

BF16 = mybir.dt.bfloat16
F32 = mybir.dt.float32
FP8 = mybir.dt.float8e4
BF16_NP = ml_dtypes.bfloat16
FP8_NP = ml_dtypes.float8_e4m3

B, S, H, NH, HD = 2, 2048, 1024, 16, 64
T = B * S               # 4096 tokens, b-major
NCORES = 8
HPC = NH // NCORES      # heads per core = 2
DPC = HPC * HD          # head dims per core = 128
KT = 128                # keys per k-tile
NKT = S // KT           # 16
QC = 512                # query chunk
NQC = S // QC           # 4
HKT = H // 128          # hidden k-tiles = 8
NDR = HKT // 2          # DoubleRow chain steps = 4
WSCALE = 16.0           # host pre-scale on q/k weights
EXPSCALE = 1.0 / np.sqrt(HD)

_CACHED = {}


def _build_nc():
    nc = bacc.Bacc(None, target_bir_lowering=False, debug=False)
    xT = nc.dram_tensor("xT", [H, T], BF16, kind="ExternalInput").ap()
    xf8 = nc.dram_tensor("xf8", [NDR, 128, 2, T], FP8, kind="ExternalInput").ap()
    wqk8 = nc.dram_tensor("wqk8", [NDR, 128, 2, 2 * 128], FP8,
                          kind="ExternalInput").ap()
    wv = nc.dram_tensor("wv", [HKT, 128, DPC], BF16, kind="ExternalInput").ap()
    woT = nc.dram_tensor("woT", [DPC, H], BF16, kind="ExternalInput").ap()
    bqk = nc.dram_tensor("bqk", [128, 2], F32, kind="ExternalInput").ap()
    vbias = nc.dram_tensor("vbias", [128, DPC], F32, kind="ExternalInput").ap()
    out = nc.dram_tensor("out", [T, H], BF16, kind="ExternalOutput").ap()
    import os
    dbg = os.environ.get("K2DEBUG") == "1"
    if dbg:
        dvaug = nc.dram_tensor("dvaug", [128, NKT, HPC, HD + 1], BF16,
                               kind="ExternalOutput").ap()
        dvt = nc.dram_tensor("dvt", [128, T], BF16, kind="ExternalOutput").ap()
        dqk = nc.dram_tensor("dqk", [128, T], BF16, kind="ExternalOutput").ap()

    EXP = mybir.ActivationFunctionType.Exp
    MULT = mybir.AluOpType.mult
    ADD = mybir.AluOpType.add
    DR = mybir.MatmulPerfMode.DoubleRow

    with tile.TileContext(nc) as tc:
        with (
            tc.tile_pool(name="const", bufs=1) as constp,
            tc.tile_pool(name="xw", bufs=1) as xwp,
            tc.tile_pool(name="qkv", bufs=1) as qkvp,
            tc.tile_pool(name="vaug", bufs=1) as vaugp,
            tc.tile_pool(name="oT", bufs=2) as oTp,
            tc.tile_pool(name="p", bufs=4) as pp,
            tc.tile_pool(name="ysb", bufs=3) as ysbp,
            tc.tile_pool(name="recip", bufs=2) as recipp,
            tc.tile_pool(name="rep", bufs=2) as repp,
            tc.tile_pool(name="ps", bufs=2, space="PSUM") as psp,
            tc.tile_pool(name="pso", bufs=2, space="PSUM") as psop,
        ):
            # ---- weights + x in; b0 halves first (v4 schedule) ----
            bias_sb = constp.tile([128, 2], F32, tag="bias")
            nc.sync.dma_start(bias_sb[:], bqk[:])
            vbias_sb = constp.tile([128, DPC], F32, tag="vbias")
            nc.sync.dma_start(vbias_sb[:], vbias[:])
            wo_sb = constp.tile([DPC, H], BF16, tag="wo")
            nc.sync.dma_start(wo_sb[:], woT[:])
            ones_sb = constp.tile([128, 128], BF16, tag="ones")
            nc.vector.memset(ones_sb[:], 1.0)

            wqk_sb = [constp.tile([128, 2, 256], FP8, name=f"wqk{s}", tag=f"wqk{s}")
                      for s in range(NDR)]
            for s in range(NDR):
                nc.sync.dma_start(wqk_sb[s][:], wqk8[s])
            wv_sb = [constp.tile([128, DPC], BF16, name=f"wv{k}", tag=f"wv{k}")
                     for k in range(HKT)]
            for k in range(HKT):
                nc.sync.dma_start(wv_sb[k][:], wv[k])

            xf8_sb = [xwp.tile([128, 2, T], FP8, name=f"xf8{s}", tag=f"xf8{s}")
                      for s in range(NDR)]
            xT_sb = [xwp.tile([128, T], BF16, name=f"xsb{k}", tag=f"x{k}")
                     for k in range(HKT)]
            for half in range(2):
                sl = slice(half * S, (half + 1) * S)
                for s in range(NDR):
                    eng = nc.sync if s % 2 == 0 else nc.scalar
                    eng.dma_start(xf8_sb[s][:, :, sl], xf8[s][:, :, sl])
                for k in range(HKT):
                    eng = nc.sync if k % 2 == 0 else nc.scalar
                    eng.dma_start(xT_sb[k][:, sl], xT[k * 128:(k + 1) * 128, sl])

            # vaug tiles (memset to 1.0 so the ones columns are ready)
            vaug = {}
            for b in range(B):
                va = vaugp.tile([128, NKT, HPC, HD + 1], BF16,
                                name=f"va{b}", tag=f"va{b}")
                nc.vector.memset(va[:], 1.0)
                vaug[b] = va

            qkvT_sb = {
                fg: qkvp.tile([128, T], BF16, name=f"qkvsb{fg}", tag=f"qkv{fg}")
                for fg in range(2)
            }

            # ---- projection steps ----
            def v_tile_step(b, kt):
                def run():
                    v_ps = psp.tile([128, DPC], F32, name=f"vps{b}{kt}", tag="y")
                    for k in range(HKT):
                        nc.tensor.matmul(
                            v_ps[:],
                            lhsT=xT_sb[k][:, b * S + kt * KT:b * S + (kt + 1) * KT],
                            rhs=wv_sb[k][:],
                            start=(k == 0), stop=(k == HKT - 1),
                        )
                    nc.vector.tensor_add(
                        vaug[b][:, kt, :, 0:HD],
                        v_ps[:].rearrange("p (j d) -> p j d", j=HPC),
                        vbias_sb[:].rearrange("p (j d) -> p j d", j=HPC),
                    )
                return run

            def qk_chunk_step(fg, half, t):
                # fp8 DoubleRow chain: 4 steps of 256-deep contraction
                def run():
                    ps = psp.tile([128, 512], F32, name=f"qkc{fg}{half}{t}", tag="y")
                    for s in range(NDR):
                        nc.tensor.matmul(
                            ps[:],
                            lhsT=wqk_sb[s][:, :, fg * 128:(fg + 1) * 128],
                            rhs=xf8_sb[s][:, :,
                                          half * S + t * 512:half * S + (t + 1) * 512],
                            start=(s == 0), stop=(s == NDR - 1),
                            perf_mode=DR,
                        )
                    nc.vector.tensor_scalar(
                        out=qkvT_sb[fg][:, half * S + t * 512:half * S + (t + 1) * 512],
                        in0=ps[:],
                        scalar1=1.0 / WSCALE,
                        scalar2=bias_sb[:, fg:fg + 1],
                        op0=MULT, op1=ADD,
                    )
                return run

            # ---- b0 projections up front ----
            for fg in (0, 1):
                for t in range(4):
                    qk_chunk_step(fg, 0, t)()

            qT_sb, kT_sb = qkvT_sb[0], qkvT_sb[1]

            # ---- attention + pipelined epilogue ----
            pending = [v_tile_step(0, kt) for kt in range(NKT)]
            extra = [v_tile_step(1, kt) for kt in range(NKT)]
            # order the b1 q/k chunks so the ones b1-qc0 scores need come first
            extra += [qk_chunk_step(fg, 1, t)
                      for fg, t in ((1, 0), (0, 0), (1, 1), (1, 2), (1, 3),
                                    (0, 1), (0, 2), (0, 3))]

            def make_epilogue(b, qc, oT_raw, csrow):
                q0 = b * S + qc * QC
                state = {}

                def bcast_step():
                    # broadcast the bf16 colsum row across partitions via
                    # K=1 ones-matmuls, then 1/x on the [128,512] tiles
                    # (reciprocal_approx_fast doubles as the PSUM->SBUF copy;
                    # it is broken on 1-partition APs, fine on 128)
                    csA_ps = psp.tile([128, 512], F32, name=f"rA{b}{qc}", tag="y")
                    csB_ps = psp.tile([128, 512], F32, name=f"rB{b}{qc}", tag="y")
                    nc.tensor.matmul(
                        csA_ps[:], lhsT=ones_sb[64:65, :],
                        rhs=csrow[64:65, 0:512], start=True, stop=True,
                    )
                    nc.tensor.matmul(
                        csB_ps[:], lhsT=ones_sb[64:65, :],
                        rhs=csrow[64:65, 512:1024], start=True, stop=True,
                    )
                    repA = repp.tile([128, 512], F32, name=f"rsA{b}{qc}", tag="repA")
                    repB = repp.tile([128, 512], F32, name=f"rsB{b}{qc}", tag="repB")
                    nc.vector.reciprocal_approx_fast(repA[:], csA_ps[:])
                    nc.vector.reciprocal_approx_fast(repB[:], csB_ps[:])
                    state["repA"], state["repB"] = repA, repB

                def norm_step():
                    oT = oTp.tile([128, QC], BF16, name=f"oT{b}{qc}", tag="oT")
                    nc.vector.tensor_tensor(
                        oT[0:64, :], oT_raw[0:64, :], state["repA"][0:64, :],
                        op=MULT,
                    )
                    nc.vector.tensor_tensor(
                        oT[64:128, :], oT_raw[64:128, :], state["repB"][64:128, :],
                        op=MULT,
                    )
                    state["oT"] = oT

                def y_step(tt, ec):
                    def run():
                        y_ps = psp.tile([128, 512], F32,
                                        name=f"y{b}{qc}{tt}{ec}", tag="y")
                        nc.tensor.matmul(
                            y_ps[:],
                            lhsT=state["oT"][:, tt * KT:(tt + 1) * KT],
                            rhs=wo_sb[:, ec * 512:(ec + 1) * 512],
                            start=True, stop=True,
                        )
                        y_sb = ysbp.tile([128, 512], BF16,
                                         name=f"ys{b}{qc}{tt}{ec}", tag="ysb")
                        nc.vector.tensor_copy(y_sb[:], y_ps[:])
                        nc.gpsimd.dma_start(
                            out[q0 + tt * KT:q0 + (tt + 1) * KT,
                                ec * 512:(ec + 1) * 512],
                            y_sb[:],
                        )
                    return run

                steps = [bcast_step, norm_step]
                for tt in range(4):
                    for ec in range(2):
                        steps.append(y_step(tt, ec))
                return steps

            for b in range(B):
                for qc in range(NQC):
                    q0 = b * S + qc * QC
                    o_ps = [psop.tile([HD + 1, QC], F32,
                                      name=f"o{b}{qc}{h}", tag="o")
                            for h in range(HPC)]
                    p_tiles = []
                    for kt in range(NKT):
                        s_ps = psp.tile([128, HPC * QC], F32, tag="s")
                        for h in range(HPC):
                            nc.tensor.matmul(
                                s_ps[:, h * QC:(h + 1) * QC],
                                lhsT=kT_sb[h * HD:(h + 1) * HD,
                                           b * S + kt * KT:b * S + (kt + 1) * KT],
                                rhs=qT_sb[h * HD:(h + 1) * HD, q0:q0 + QC],
                                start=True, stop=True,
                            )
                        p_sb = pp.tile([128, HPC * QC], BF16, tag="p")
                        nc.scalar.activation(p_sb[:], s_ps[:], EXP,
                                             scale=float(EXPSCALE))
                        p_tiles.append(p_sb)
                        if pending:
                            pending.pop(0)()
                        if pending and len(pending) > NKT - 1 - kt:
                            pending.pop(0)()
                        if kt >= 3:
                            for h in range(HPC):
                                nc.tensor.matmul(
                                    o_ps[h][:],
                                    lhsT=vaug[b][:, kt - 3, h, :],
                                    rhs=p_tiles[kt - 3][:, h * QC:(h + 1) * QC],
                                    start=(kt - 3 == 0), stop=False,
                                )
                    for lag in (NKT - 3, NKT - 2, NKT - 1):
                        for h in range(HPC):
                            nc.tensor.matmul(
                                o_ps[h][:],
                                lhsT=vaug[b][:, lag, h, :],
                                rhs=p_tiles[lag][:, h * QC:(h + 1) * QC],
                                start=False, stop=(lag == NKT - 1),
                            )
                    # free o_ps fast: raw o to SBUF bf16 + colsum rows
                    # (row 64) to a bf16 SBUF row for the broadcast matmul
                    oT_raw = oTp.tile([128, QC], BF16,
                                      name=f"oR{b}{qc}", tag="oTraw")
                    nc.vector.tensor_copy(oT_raw[0:64, :], o_ps[0][0:64, :])
                    nc.vector.tensor_copy(oT_raw[64:128, :], o_ps[1][0:64, :])
                    csrow = recipp.tile([128, 2 * QC], BF16,
                                        name=f"rc{b}{qc}", tag="recip")
                    nc.vector.tensor_copy(csrow[64:65, 0:512],
                                          o_ps[0][64:65, :])
                    nc.vector.tensor_copy(csrow[64:65, 512:1024],
                                          o_ps[1][64:65, :])
                    while pending:
                        pending.pop(0)()
                    epi = make_epilogue(b, qc, oT_raw, csrow)
                    # 18 > NKT: the 2-pops-per-kt drain rule clears the
                    # backlog within one chunk; ensures all of `extra` is
                    # emitted during b0 so b1 scores never lead their inputs.
                    take = min(len(extra), 18 - len(epi))
                    pending = extra[:take] + epi
                    del extra[:take]
            while pending:
                pending.pop(0)()
            if dbg:
                nc.sync.dma_start(dvaug[:], vaug[0][:])
                nc.sync.dma_start(dvt[:], vT_sb[:])
                nc.sync.dma_start(dqk[:], qkvT_sb[1][:])
    nc.compile()
    return nc


def _get_nc():
    if "nc" not in _CACHED:
        _CACHED["nc"] = _build_nc()
    return _CACHED["nc"]


def _host_prep(x, qkv_w, qkv_b, out_w):
    x = np.asarray(x, dtype=np.float32)
    qkv_w = np.asarray(qkv_w, dtype=np.float32)
    qkv_b = np.asarray(qkv_b, dtype=np.float32)
    out_w = np.asarray(out_w, dtype=np.float32)

    xTf = np.ascontiguousarray(x.reshape(T, H).T)
    xT = xTf.astype(BF16_NP)
    # fp8 x in DoubleRow layout: [NDR, 128, 2, T]
    xf8 = np.ascontiguousarray(
        xTf.reshape(NDR, 2, 128, T).transpose(0, 2, 1, 3)
    ).astype(FP8_NP)

    in_maps = []
    for c in range(NCORES):
        wq = qkv_w[128 * c:128 * c + 128]          # [128, H]
        wk = qkv_w[H + 128 * c:H + 128 * c + 128]
        wv_c = qkv_w[2 * H + 128 * c:2 * H + 128 * c + 128]
        # wqk8: [NDR, 128, 2, 256]; cols 0-127 = q feats, 128-255 = k feats
        wqk = np.concatenate([wq, wk], 0).T * WSCALE   # [H, 256]
        wqk8 = np.ascontiguousarray(
            wqk.reshape(NDR, 2, 128, 256).transpose(0, 2, 1, 3)
        ).astype(FP8_NP)
        # wv: [HKT, 128, DPC] = hidden-major slices of wv_c.T
        wvT = wv_c.T                                    # [H, DPC]
        wv8 = np.ascontiguousarray(
            wvT.reshape(HKT, 128, DPC)).astype(BF16_NP)
        woTa = np.ascontiguousarray(
            out_w[:, DPC * c:DPC * (c + 1)].T).astype(BF16_NP)  # [DPC, H]
        bqk = np.stack(
            [qkv_b[fg * H + 128 * c:fg * H + 128 * c + 128] for fg in range(2)],
            axis=1,
        ).astype(np.float32)
        vb = np.broadcast_to(
            qkv_b[2 * H + 128 * c:2 * H + 128 * c + 128][None, :], (128, DPC)
        ).astype(np.float32)
        in_maps.append({
            "xT": xT,
            "xf8": xf8,
            "wqk8": wqk8,
            "wv": wv8,
            "woT": woTa,
            "bqk": np.ascontiguousarray(bqk),
            "vbias": np.ascontiguousarray(vb),
        })
    return in_maps


def _run(in_maps, trace=False):
    if trace and "antenv.axon_hooks" not in sys.modules:
        try:
            import trn_agent_boot.trn_boot as _tb
            _hook = _tb._ntff_profile_via_ctypes("/opt/axon/libaxon_pjrt.so")
            _m = types.ModuleType("antenv.axon_hooks")
            _m.get_axon_ntff_profile_hook = lambda: _hook
            sys.modules["antenv.axon_hooks"] = _m
        except Exception:
            trace = False
    from concourse.bass_utils import run_bass_kernel_spmd

    nc = _get_nc()
    res = run_bass_kernel_spmd(nc, in_maps, core_ids=list(range(NCORES)), trace=trace)
    return res


def kernel(x, qkv_w, qkv_b, out_w, out_b):
    in_maps = _host_prep(x, qkv_w, qkv_b, out_w)
    res = _run(in_maps, trace=False)
    total = np.zeros((T, H), np.float32)
    for c in range(NCORES):
        total += res.results[c]["out"].astype(np.float32)
    total += np.asarray(out_b, dtype=np.float32)[None, :]
    return total.reshape(B, S, H)


# revision 4
# speedup vs baseline: 1.2545x; 1.0107x over previous
"""Fused multi-head attention on 8 Trainium2 NeuronCores — v2.

vs v1 baseline:
  - q/k projection in fp8 (e4m3) DoubleRow mode: 4-step chains at 2x rate.
    Weights pre-scaled by 16 on host (into e4m3 normal range); 1/16 folded
    into the bias tensor_scalar.
  - Attention output o is pre-normalized (divide by softmax colsum) BEFORE
    the out-projection, so out-proj is a single K=128 matmul over both
    heads and the huge per-tile DVE rescale epilogue disappears.
    colsum rides the PV matmul as row 64 ([v|ones] aug layout); recip on
    DVE (bf16), broadcast across partitions via a tiny K=1 ones-matmul on
    the PE, normalized with one tensor_tensor per head half.
  - Output written bf16 (host sums partials in f32).

Layouts per core (2 heads A=2c, B=2c+1):
  xT    [128,4096]x8  bf16   hidden on partitions, tokens free (b-major)
  xf8   [128,2,4096]x4 fp8   DoubleRow pairs of hidden k-subtiles
  qkvT  [128,4096]x2  bf16   q/k feature-major (A dims 0-63, B 64-127)
  vaug  [128,16,2,65] bf16   per kt, per head: [v(64) | ones(1)]
  o_ps  [65,512]x2    PSUM   rows 0-63 = o, row 64 = colsum
  oT    [128,512]     bf16   normalized, A on parts 0-63, B on 64-127
  y     [128,512]     PSUM   K=128 out-proj; copied to bf16, DMA'd out
"""

import sys
import types
import numpy as np
import ml_dtypes

import concourse.bass as bass
import concourse.tile as tile
from concourse import bacc, mybir

BF16 = mybir.dt.bfloat16
F32 = mybir.dt.float32
FP8 = mybir.dt.float8e4
BF16_NP = ml_dtypes.bfloat16
FP8_NP = ml_dtypes.float8_e4m3

B, S, H, NH, HD = 2, 2048, 1024, 16, 64
T = B * S               # 4096 tokens, b-major
NCORES = 8
HPC = NH // NCORES      # heads per core = 2
DPC = HPC * HD          # head dims per core = 128
KT = 128                # keys per k-tile
NKT = S // KT           # 16
QC = 512                # query chunk
NQC = S // QC           # 4
HKT = H // 128          # hidden k-tiles = 8
NDR = HKT // 2          # DoubleRow chain steps = 4
WSCALE = 16.0           # host pre-scale on q/k weights
EXPSCALE = 1.0 / np.sqrt(HD)

_CACHED = {}


def _build_nc():
    nc = bacc.Bacc(None, target_bir_lowering=False, debug=False)
    xT = nc.dram_tensor("xT", [H, T], BF16, kind="ExternalInput").ap()
    xf8 = nc.dram_tensor("xf8", [NDR, 128, 2, T], FP8, kind="ExternalInput").ap()
    wqk8 = nc.dram_tensor("wqk8", [NDR, 128, 2, 2 * 128], FP8,
                          kind="ExternalInput").ap()
    wv = nc.dram_tensor("wv", [HKT, 128, DPC], BF16, kind="ExternalInput").ap()
    woT = nc.dram_tensor("woT", [DPC, H], BF16, kind="ExternalInput").ap()
    bqk = nc.dram_tensor("bqk", [128, 2], F32, kind="ExternalInput").ap()
    vbias = nc.dram_tensor("vbias", [128, DPC], F32, kind="ExternalInput").ap()
    out = nc.dram_tensor("out", [T, H], BF16, kind="ExternalOutput").ap()
    import os
    dbg = os.environ.get("K2DEBUG") == "1"
    if dbg:
        dvaug = nc.dram_tensor("dvaug", [128, NKT, HPC, HD + 1], BF16,
                               kind="ExternalOutput").ap()
        dvt = nc.dram_tensor("dvt", [128, T], BF16, kind="ExternalOutput").ap()
        dqk = nc.dram_tensor("dqk", [128, T], BF16, kind="ExternalOutput").ap()

    EXP = mybir.ActivationFunctionType.Exp
    MULT = mybir.AluOpType.mult
    ADD = mybir.AluOpType.add
    DR = mybir.MatmulPerfMode.DoubleRow

    with tile.TileContext(nc) as tc:
        with (
            tc.tile_pool(name="const", bufs=1) as constp,
            tc.tile_pool(name="xw", bufs=1) as xwp,
            tc.tile_pool(name="qkv", bufs=1) as qkvp,
            tc.tile_pool(name="vaug", bufs=1) as vaugp,
            tc.tile_pool(name="oT", bufs=2) as oTp,
            tc.tile_pool(name="p", bufs=5) as pp,
            tc.tile_pool(name="ysb", bufs=4) as ysbp,
            tc.tile_pool(name="recip", bufs=2) as recipp,
            tc.tile_pool(name="rep", bufs=2) as repp,
            tc.tile_pool(name="ps", bufs=2, space="PSUM") as psp,
            tc.tile_pool(name="pso", bufs=2, space="PSUM") as psop,
        ):
            # ---- weights + x in; b0 halves first (v4 schedule) ----
            bias_sb = constp.tile([128, 2], F32, tag="bias")
            nc.sync.dma_start(bias_sb[:], bqk[:])
            vbias_sb = constp.tile([128, DPC], F32, tag="vbias")
            nc.sync.dma_start(vbias_sb[:], vbias[:])
            wo_sb = constp.tile([DPC, H], BF16, tag="wo")
            nc.sync.dma_start(wo_sb[:], woT[:])
            ones_sb = constp.tile([128, 128], BF16, tag="ones")
            nc.vector.memset(ones_sb[:], 1.0)

            wqk_sb = [constp.tile([128, 2, 256], FP8, name=f"wqk{s}", tag=f"wqk{s}")
                      for s in range(NDR)]
            for s in range(NDR):
                nc.sync.dma_start(wqk_sb[s][:], wqk8[s])
            wv_sb = [constp.tile([128, DPC], BF16, name=f"wv{k}", tag=f"wv{k}")
                     for k in range(HKT)]
            for k in range(HKT):
                nc.sync.dma_start(wv_sb[k][:], wv[k])

            xf8_sb = [xwp.tile([128, 2, T], FP8, name=f"xf8{s}", tag=f"xf8{s}")
                      for s in range(NDR)]
            xT_sb = [xwp.tile([128, T], BF16, name=f"xsb{k}", tag=f"x{k}")
                     for k in range(HKT)]
            for half in range(2):
                sl = slice(half * S, (half + 1) * S)
                for s in range(NDR):
                    eng = nc.sync if s % 2 == 0 else nc.scalar
                    eng.dma_start(xf8_sb[s][:, :, sl], xf8[s][:, :, sl])
                for k in range(HKT):
                    eng = nc.sync if k % 2 == 0 else nc.scalar
                    eng.dma_start(xT_sb[k][:, sl], xT[k * 128:(k + 1) * 128, sl])

            # vaug tiles (memset to 1.0 so the ones columns are ready)
            vaug = {}
            for b in range(B):
                va = vaugp.tile([128, NKT, HPC, HD + 1], BF16,
                                name=f"va{b}", tag=f"va{b}")
                nc.vector.memset(va[:], 1.0)
                vaug[b] = va

            qkvT_sb = {
                fg: qkvp.tile([128, T], BF16, name=f"qkvsb{fg}", tag=f"qkv{fg}")
                for fg in range(2)
            }

            # ---- projection steps ----
            def v_tile_step(b, kt):
                def run():
                    v_ps = psp.tile([128, DPC], F32, name=f"vps{b}{kt}", tag="y")
                    for k in range(HKT):
                        nc.tensor.matmul(
                            v_ps[:],
                            lhsT=xT_sb[k][:, b * S + kt * KT:b * S + (kt + 1) * KT],
                            rhs=wv_sb[k][:],
                            start=(k == 0), stop=(k == HKT - 1),
                        )
                    nc.vector.tensor_add(
                        vaug[b][:, kt, :, 0:HD],
                        v_ps[:].rearrange("p (j d) -> p j d", j=HPC),
                        vbias_sb[:].rearrange("p (j d) -> p j d", j=HPC),
                    )
                return run

            def qk_chunk_step(fg, half, t):
                # fp8 DoubleRow chain: 4 steps of 256-deep contraction
                def run():
                    ps = psp.tile([128, 512], F32, name=f"qkc{fg}{half}{t}", tag="y")
                    for s in range(NDR):
                        nc.tensor.matmul(
                            ps[:],
                            lhsT=wqk_sb[s][:, :, fg * 128:(fg + 1) * 128],
                            rhs=xf8_sb[s][:, :,
                                          half * S + t * 512:half * S + (t + 1) * 512],
                            start=(s == 0), stop=(s == NDR - 1),
                            perf_mode=DR,
                        )
                    nc.vector.tensor_scalar(
                        out=qkvT_sb[fg][:, half * S + t * 512:half * S + (t + 1) * 512],
                        in0=ps[:],
                        scalar1=1.0 / WSCALE,
                        scalar2=bias_sb[:, fg:fg + 1],
                        op0=MULT, op1=ADD,
                    )
                return run

            # ---- b0 projections up front ----
            for fg in (0, 1):
                for t in range(4):
                    qk_chunk_step(fg, 0, t)()

            qT_sb, kT_sb = qkvT_sb[0], qkvT_sb[1]

            # ---- attention + pipelined epilogue ----
            pending = [v_tile_step(0, kt) for kt in range(NKT)]
            extra = [v_tile_step(1, kt) for kt in range(NKT)]
            # order the b1 q/k chunks so the ones b1-qc0 scores need come first
            extra += [qk_chunk_step(fg, 1, t)
                      for fg, t in ((1, 0), (0, 0), (1, 1), (1, 2), (1, 3),
                                    (0, 1), (0, 2), (0, 3))]

            def make_epilogue(b, qc, oT_raw, csrow):
                q0 = b * S + qc * QC
                state = {}

                def bcast_step():
                    # broadcast the bf16 colsum row across partitions via
                    # K=1 ones-matmuls, then 1/x on the [128,512] tiles
                    # (reciprocal_approx_fast doubles as the PSUM->SBUF copy;
                    # it is broken on 1-partition APs, fine on 128)
                    csA_ps = psp.tile([128, 512], F32, name=f"rA{b}{qc}", tag="y")
                    csB_ps = psp.tile([128, 512], F32, name=f"rB{b}{qc}", tag="y")
                    nc.tensor.matmul(
                        csA_ps[:], lhsT=ones_sb[64:65, :],
                        rhs=csrow[64:65, 0:512], start=True, stop=True,
                    )
                    nc.tensor.matmul(
                        csB_ps[:], lhsT=ones_sb[64:65, :],
                        rhs=csrow[64:65, 512:1024], start=True, stop=True,
                    )
                    repA = repp.tile([128, 512], F32, name=f"rsA{b}{qc}", tag="repA")
                    repB = repp.tile([128, 512], F32, name=f"rsB{b}{qc}", tag="repB")
                    nc.vector.reciprocal_approx_fast(repA[:], csA_ps[:])
                    nc.vector.reciprocal_approx_fast(repB[:], csB_ps[:])
                    state["repA"], state["repB"] = repA, repB

                def norm_step():
                    oT = oTp.tile([128, QC], BF16, name=f"oT{b}{qc}", tag="oT")
                    nc.vector.tensor_tensor(
                        oT[0:64, :], oT_raw[0:64, :], state["repA"][0:64, :],
                        op=MULT,
                    )
                    nc.vector.tensor_tensor(
                        oT[64:128, :], oT_raw[64:128, :], state["repB"][64:128, :],
                        op=MULT,
                    )
                    state["oT"] = oT

                def y_step(tt, ec):
                    def run():
                        y_ps = psp.tile([128, 512], F32,
                                        name=f"y{b}{qc}{tt}{ec}", tag="y")
                        nc.tensor.matmul(
                            y_ps[:],
                            lhsT=state["oT"][:, tt * KT:(tt + 1) * KT],
                            rhs=wo_sb[:, ec * 512:(ec + 1) * 512],
                            start=True, stop=True,
                        )
                        y_sb = ysbp.tile([128, 512], BF16,
                                         name=f"ys{b}{qc}{tt}{ec}", tag="ysb")
                        nc.vector.tensor_copy(y_sb[:], y_ps[:])
                        nc.gpsimd.dma_start(
                            out[q0 + tt * KT:q0 + (tt + 1) * KT,
                                ec * 512:(ec + 1) * 512],
                            y_sb[:],
                        )
                    return run

                steps = [bcast_step, norm_step]
                for tt in range(4):
                    for ec in range(2):
                        steps.append(y_step(tt, ec))
                return steps

            for b in range(B):
                for qc in range(NQC):
                    q0 = b * S + qc * QC
                    o_ps = [psop.tile([HD + 1, QC], F32,
                                      name=f"o{b}{qc}{h}", tag="o")
                            for h in range(HPC)]
                    p_tiles = []
                    for kt in range(NKT):
                        s_ps = psp.tile([128, HPC * QC], F32, tag="s")
                        for h in range(HPC):
                            nc.tensor.matmul(
                                s_ps[:, h * QC:(h + 1) * QC],
                                lhsT=kT_sb[h * HD:(h + 1) * HD,
                                           b * S + kt * KT:b * S + (kt + 1) * KT],
                                rhs=qT_sb[h * HD:(h + 1) * HD, q0:q0 + QC],
                                start=True, stop=True,
                            )
                        p_sb = pp.tile([128, HPC * QC], BF16, tag="p")
                        nc.scalar.activation(p_sb[:], s_ps[:], EXP,
                                             scale=float(EXPSCALE))
                        p_tiles.append(p_sb)
                        if pending:
                            pending.pop(0)()
                        if pending and len(pending) > NKT - 1 - kt:
                            pending.pop(0)()
                        if kt >= 4:
                            for h in range(HPC):
                                nc.tensor.matmul(
                                    o_ps[h][:],
                                    lhsT=vaug[b][:, kt - 4, h, :],
                                    rhs=p_tiles[kt - 4][:, h * QC:(h + 1) * QC],
                                    start=(kt - 4 == 0), stop=False,
                                )
                    for lag in (NKT - 4, NKT - 3, NKT - 2, NKT - 1):
                        for h in range(HPC):
                            nc.tensor.matmul(
                                o_ps[h][:],
                                lhsT=vaug[b][:, lag, h, :],
                                rhs=p_tiles[lag][:, h * QC:(h + 1) * QC],
                                start=False, stop=(lag == NKT - 1),
                            )
                    # free o_ps fast: raw o to SBUF bf16 + colsum rows
                    # (row 64) to a bf16 SBUF row for the broadcast matmul
                    oT_raw = oTp.tile([128, QC], BF16,
                                      name=f"oR{b}{qc}", tag="oTraw")
                    nc.vector.tensor_copy(oT_raw[0:64, :], o_ps[0][0:64, :])
                    nc.vector.tensor_copy(oT_raw[64:128, :], o_ps[1][0:64, :])
                    csrow = recipp.tile([128, 2 * QC], BF16,
                                        name=f"rc{b}{qc}", tag="recip")
                    nc.vector.tensor_copy(csrow[64:65, 0:512],
                                          o_ps[0][64:65, :])
                    nc.vector.tensor_copy(csrow[64:65, 512:1024],
                                          o_ps[1][64:65, :])
                    while pending:
                        pending.pop(0)()
                    epi = make_epilogue(b, qc, oT_raw, csrow)
                    # 18 > NKT: the 2-pops-per-kt drain rule clears the
                    # backlog within one chunk; ensures all of `extra` is
                    # emitted during b0 so b1 scores never lead their inputs.
                    take = min(len(extra), 18 - len(epi))
                    pending = extra[:take] + epi
                    del extra[:take]
            while pending:
                pending.pop(0)()
            if dbg:
                nc.sync.dma_start(dvaug[:], vaug[0][:])
                nc.sync.dma_start(dvt[:], vT_sb[:])
                nc.sync.dma_start(dqk[:], qkvT_sb[1][:])
    nc.compile()
    return nc


def _get_nc():
    if "nc" not in _CACHED:
        _CACHED["nc"] = _build_nc()
    return _CACHED["nc"]


def _host_prep(x, qkv_w, qkv_b, out_w):
    x = np.asarray(x, dtype=np.float32)
    qkv_w = np.asarray(qkv_w, dtype=np.float32)
    qkv_b = np.asarray(qkv_b, dtype=np.float32)
    out_w = np.asarray(out_w, dtype=np.float32)

    xTf = np.ascontiguousarray(x.reshape(T, H).T)
    xT = xTf.astype(BF16_NP)
    # fp8 x in DoubleRow layout: [NDR, 128, 2, T]
    xf8 = np.ascontiguousarray(
        xTf.reshape(NDR, 2, 128, T).transpose(0, 2, 1, 3)
    ).astype(FP8_NP)

    in_maps = []
    for c in range(NCORES):
        wq = qkv_w[128 * c:128 * c + 128]          # [128, H]
        wk = qkv_w[H + 128 * c:H + 128 * c + 128]
        wv_c = qkv_w[2 * H + 128 * c:2 * H + 128 * c + 128]
        # wqk8: [NDR, 128, 2, 256]; cols 0-127 = q feats, 128-255 = k feats
        wqk = np.concatenate([wq, wk], 0).T * WSCALE   # [H, 256]
        wqk8 = np.ascontiguousarray(
            wqk.reshape(NDR, 2, 128, 256).transpose(0, 2, 1, 3)
        ).astype(FP8_NP)
        # wv: [HKT, 128, DPC] = hidden-major slices of wv_c.T
        wvT = wv_c.T                                    # [H, DPC]
        wv8 = np.ascontiguousarray(
            wvT.reshape(HKT, 128, DPC)).astype(BF16_NP)
        woTa = np.ascontiguousarray(
            out_w[:, DPC * c:DPC * (c + 1)].T).astype(BF16_NP)  # [DPC, H]
        bqk = np.stack(
            [qkv_b[fg * H + 128 * c:fg * H + 128 * c + 128] for fg in range(2)],
            axis=1,
        ).astype(np.float32)
        vb = np.broadcast_to(
            qkv_b[2 * H + 128 * c:2 * H + 128 * c + 128][None, :], (128, DPC)
        ).astype(np.float32)
        in_maps.append({
            "xT": xT,
            "xf8": xf8,
            "wqk8": wqk8,
            "wv": wv8,
            "woT": woTa,
            "bqk": np.ascontiguousarray(bqk),
            "vbias": np.ascontiguousarray(vb),
        })
    return in_maps


def _run(in_maps, trace=False):
    if trace and "antenv.axon_hooks" not in sys.modules:
        try:
            import trn_agent_boot.trn_boot as _tb
            _hook = _tb._ntff_profile_via_ctypes("/opt/axon/libaxon_pjrt.so")
            _m = types.ModuleType("antenv.axon_hooks")
            _m.get_axon_ntff_profile_hook = lambda: _hook
            sys.modules["antenv.axon_hooks"] = _m
        except Exception:
            trace = False
    from concourse.bass_utils import run_bass_kernel_spmd

    nc = _get_nc()
    res = run_bass_kernel_spmd(nc, in_maps, core_ids=list(range(NCORES)), trace=trace)
    return res


def kernel(x, qkv_w, qkv_b, out_w, out_b):
    in_maps = _host_prep(x, qkv_w, qkv_b, out_w)
    res = _run(in_maps, trace=False)
    total = np.zeros((T, H), np.float32)
    for c in range(NCORES):
        total += res.results[c]["out"].astype(np.float32)
    total += np.asarray(out_b, dtype=np.float32)[None, :]
    return total.reshape(B, S, H)


# revision 5
# speedup vs baseline: 1.2584x; 1.0031x over previous
"""Fused multi-head attention on 8 Trainium2 NeuronCores — v2.

vs v1 baseline:
  - q/k projection in fp8 (e4m3) DoubleRow mode: 4-step chains at 2x rate.
    Weights pre-scaled by 16 on host (into e4m3 normal range); 1/16 folded
    into the bias tensor_scalar.
  - Attention output o is pre-normalized (divide by softmax colsum) BEFORE
    the out-projection, so out-proj is a single K=128 matmul over both
    heads and the huge per-tile DVE rescale epilogue disappears.
    colsum rides the PV matmul as row 64 ([v|ones] aug layout); recip on
    DVE (bf16), broadcast across partitions via a tiny K=1 ones-matmul on
    the PE, normalized with one tensor_tensor per head half.
  - Output written bf16 (host sums partials in f32).

Layouts per core (2 heads A=2c, B=2c+1):
  xT    [128,4096]x8  bf16   hidden on partitions, tokens free (b-major)
  xf8   [128,2,4096]x4 fp8   DoubleRow pairs of hidden k-subtiles
  qkvT  [128,4096]x2  bf16   q/k feature-major (A dims 0-63, B 64-127)
  vaug  [128,16,2,65] bf16   per kt, per head: [v(64) | ones(1)]
  o_ps  [65,512]x2    PSUM   rows 0-63 = o, row 64 = colsum
  oT    [128,512]     bf16   normalized, A on parts 0-63, B on 64-127
  y     [128,512]     PSUM   K=128 out-proj; copied to bf16, DMA'd out
"""

import sys
import types
import numpy as np
import ml_dtypes

import concourse.bass as bass
import concourse.tile as tile
from concourse import bacc, mybir

BF16 = mybir.dt.bfloat16
F32 = mybir.dt.float32
FP8 = mybir.dt.float8e4
BF16_NP = ml_dtypes.bfloat16
FP8_NP = ml_dtypes.float8_e4m3

B, S, H, NH, HD = 2, 2048, 1024, 16, 64
T = B * S               # 4096 tokens, b-major
NCORES = 8
HPC = NH // NCORES      # heads per core = 2
DPC = HPC * HD          # head dims per core = 128
KT = 128                # keys per k-tile
NKT = S // KT           # 16
QC = 512                # query chunk
NQC = S // QC           # 4
HKT = H // 128          # hidden k-tiles = 8
NDR = HKT // 2          # DoubleRow chain steps = 4
WSCALE = 16.0           # host pre-scale on q/k weights
EXPSCALE = 1.0 / np.sqrt(HD)

_CACHED = {}


def _build_nc():
    nc = bacc.Bacc(None, target_bir_lowering=False, debug=False)
    xT = nc.dram_tensor("xT", [H, T], BF16, kind="ExternalInput").ap()
    xf8 = nc.dram_tensor("xf8", [NDR, 128, 2, T], FP8, kind="ExternalInput").ap()
    wqk8 = nc.dram_tensor("wqk8", [NDR, 128, 2, 2 * 128], FP8,
                          kind="ExternalInput").ap()
    wv = nc.dram_tensor("wv", [HKT, 128, DPC], BF16, kind="ExternalInput").ap()
    woT = nc.dram_tensor("woT", [DPC, H], BF16, kind="ExternalInput").ap()
    bqk = nc.dram_tensor("bqk", [128, 2], F32, kind="ExternalInput").ap()
    vbias = nc.dram_tensor("vbias", [128, DPC], F32, kind="ExternalInput").ap()
    out = nc.dram_tensor("out", [T, H], BF16, kind="ExternalOutput").ap()
    import os
    dbg = os.environ.get("K2DEBUG") == "1"
    if dbg:
        dvaug = nc.dram_tensor("dvaug", [128, NKT, HPC, HD + 1], BF16,
                               kind="ExternalOutput").ap()
        dvt = nc.dram_tensor("dvt", [128, T], BF16, kind="ExternalOutput").ap()
        dqk = nc.dram_tensor("dqk", [128, T], BF16, kind="ExternalOutput").ap()

    EXP = mybir.ActivationFunctionType.Exp
    MULT = mybir.AluOpType.mult
    ADD = mybir.AluOpType.add
    DR = mybir.MatmulPerfMode.DoubleRow

    with tile.TileContext(nc) as tc:
        with (
            tc.tile_pool(name="const", bufs=1) as constp,
            tc.tile_pool(name="xw", bufs=1) as xwp,
            tc.tile_pool(name="qkv", bufs=1) as qkvp,
            tc.tile_pool(name="vaug", bufs=1) as vaugp,
            tc.tile_pool(name="oT", bufs=2) as oTp,
            tc.tile_pool(name="p", bufs=7) as pp,
            tc.tile_pool(name="ysb", bufs=4) as ysbp,
            tc.tile_pool(name="recip", bufs=2) as recipp,
            tc.tile_pool(name="rep", bufs=2) as repp,
            tc.tile_pool(name="ps", bufs=2, space="PSUM") as psp,
            tc.tile_pool(name="pso", bufs=2, space="PSUM") as psop,
        ):
            # ---- weights + x in; b0 halves first (v4 schedule) ----
            bias_sb = constp.tile([128, 2], F32, tag="bias")
            nc.sync.dma_start(bias_sb[:], bqk[:])
            vbias_sb = constp.tile([128, DPC], F32, tag="vbias")
            nc.sync.dma_start(vbias_sb[:], vbias[:])
            wo_sb = constp.tile([DPC, H], BF16, tag="wo")
            nc.sync.dma_start(wo_sb[:], woT[:])
            ones_sb = constp.tile([128, 128], BF16, tag="ones")
            nc.vector.memset(ones_sb[:], 1.0)

            wqk_sb = [constp.tile([128, 2, 256], FP8, name=f"wqk{s}", tag=f"wqk{s}")
                      for s in range(NDR)]
            for s in range(NDR):
                nc.sync.dma_start(wqk_sb[s][:], wqk8[s])
            wv_sb = [constp.tile([128, DPC], BF16, name=f"wv{k}", tag=f"wv{k}")
                     for k in range(HKT)]
            for k in range(HKT):
                nc.sync.dma_start(wv_sb[k][:], wv[k])

            xf8_sb = [xwp.tile([128, 2, T], FP8, name=f"xf8{s}", tag=f"xf8{s}")
                      for s in range(NDR)]
            xT_sb = [xwp.tile([128, T], BF16, name=f"xsb{k}", tag=f"x{k}")
                     for k in range(HKT)]
            for half in range(2):
                sl = slice(half * S, (half + 1) * S)
                for s in range(NDR):
                    eng = nc.sync if s % 2 == 0 else nc.scalar
                    eng.dma_start(xf8_sb[s][:, :, sl], xf8[s][:, :, sl])
                for k in range(HKT):
                    eng = nc.sync if k % 2 == 0 else nc.scalar
                    eng.dma_start(xT_sb[k][:, sl], xT[k * 128:(k + 1) * 128, sl])

            # vaug tiles (memset to 1.0 so the ones columns are ready)
            vaug = {}
            for b in range(B):
                va = vaugp.tile([128, NKT, HPC, HD + 1], BF16,
                                name=f"va{b}", tag=f"va{b}")
                nc.vector.memset(va[:], 1.0)
                vaug[b] = va

            qkvT_sb = {
                fg: qkvp.tile([128, T], BF16, name=f"qkvsb{fg}", tag=f"qkv{fg}")
                for fg in range(2)
            }

            # ---- projection steps ----
            def v_tile_step(b, kt):
                def run():
                    v_ps = psp.tile([128, DPC], F32, name=f"vps{b}{kt}", tag="y")
                    for k in range(HKT):
                        nc.tensor.matmul(
                            v_ps[:],
                            lhsT=xT_sb[k][:, b * S + kt * KT:b * S + (kt + 1) * KT],
                            rhs=wv_sb[k][:],
                            start=(k == 0), stop=(k == HKT - 1),
                        )
                    nc.vector.tensor_add(
                        vaug[b][:, kt, :, 0:HD],
                        v_ps[:].rearrange("p (j d) -> p j d", j=HPC),
                        vbias_sb[:].rearrange("p (j d) -> p j d", j=HPC),
                    )
                return run

            def qk_chunk_step(fg, half, t):
                # fp8 DoubleRow chain: 4 steps of 256-deep contraction
                def run():
                    ps = psp.tile([128, 512], F32, name=f"qkc{fg}{half}{t}", tag="y")
                    for s in range(NDR):
                        nc.tensor.matmul(
                            ps[:],
                            lhsT=wqk_sb[s][:, :, fg * 128:(fg + 1) * 128],
                            rhs=xf8_sb[s][:, :,
                                          half * S + t * 512:half * S + (t + 1) * 512],
                            start=(s == 0), stop=(s == NDR - 1),
                            perf_mode=DR,
                        )
                    nc.vector.tensor_scalar(
                        out=qkvT_sb[fg][:, half * S + t * 512:half * S + (t + 1) * 512],
                        in0=ps[:],
                        scalar1=1.0 / WSCALE,
                        scalar2=bias_sb[:, fg:fg + 1],
                        op0=MULT, op1=ADD,
                    )
                return run

            # ---- b0 projections up front ----
            for fg in (0, 1):
                for t in range(4):
                    qk_chunk_step(fg, 0, t)()

            qT_sb, kT_sb = qkvT_sb[0], qkvT_sb[1]

            # ---- attention + pipelined epilogue ----
            pending = [v_tile_step(0, kt) for kt in range(NKT)]
            extra = [v_tile_step(1, kt) for kt in range(NKT)]
            # order the b1 q/k chunks so the ones b1-qc0 scores need come first
            extra += [qk_chunk_step(fg, 1, t)
                      for fg, t in ((1, 0), (0, 0), (1, 1), (1, 2), (1, 3),
                                    (0, 1), (0, 2), (0, 3))]

            def make_epilogue(b, qc, oT_raw, csrow):
                q0 = b * S + qc * QC
                state = {}

                def bcast_step():
                    # broadcast the bf16 colsum row across partitions via
                    # K=1 ones-matmuls, then 1/x on the [128,512] tiles
                    # (reciprocal_approx_fast doubles as the PSUM->SBUF copy;
                    # it is broken on 1-partition APs, fine on 128)
                    csA_ps = psp.tile([128, 512], F32, name=f"rA{b}{qc}", tag="y")
                    csB_ps = psp.tile([128, 512], F32, name=f"rB{b}{qc}", tag="y")
                    nc.tensor.matmul(
                        csA_ps[:], lhsT=ones_sb[64:65, :],
                        rhs=csrow[64:65, 0:512], start=True, stop=True,
                    )
                    nc.tensor.matmul(
                        csB_ps[:], lhsT=ones_sb[64:65, :],
                        rhs=csrow[64:65, 512:1024], start=True, stop=True,
                    )
                    repA = repp.tile([128, 512], F32, name=f"rsA{b}{qc}", tag="repA")
                    repB = repp.tile([128, 512], F32, name=f"rsB{b}{qc}", tag="repB")
                    nc.vector.reciprocal_approx_fast(repA[:], csA_ps[:])
                    nc.vector.reciprocal_approx_fast(repB[:], csB_ps[:])
                    state["repA"], state["repB"] = repA, repB

                def norm_step():
                    oT = oTp.tile([128, QC], BF16, name=f"oT{b}{qc}", tag="oT")
                    nc.vector.tensor_tensor(
                        oT[0:64, :], oT_raw[0:64, :], state["repA"][0:64, :],
                        op=MULT,
                    )
                    nc.vector.tensor_tensor(
                        oT[64:128, :], oT_raw[64:128, :], state["repB"][64:128, :],
                        op=MULT,
                    )
                    state["oT"] = oT

                def y_step(tt, ec):
                    def run():
                        y_ps = psp.tile([128, 512], F32,
                                        name=f"y{b}{qc}{tt}{ec}", tag="y")
                        nc.tensor.matmul(
                            y_ps[:],
                            lhsT=state["oT"][:, tt * KT:(tt + 1) * KT],
                            rhs=wo_sb[:, ec * 512:(ec + 1) * 512],
                            start=True, stop=True,
                        )
                        y_sb = ysbp.tile([128, 512], BF16,
                                         name=f"ys{b}{qc}{tt}{ec}", tag="ysb")
                        nc.vector.tensor_copy(y_sb[:], y_ps[:])
                        nc.gpsimd.dma_start(
                            out[q0 + tt * KT:q0 + (tt + 1) * KT,
                                ec * 512:(ec + 1) * 512],
                            y_sb[:],
                        )
                    return run

                steps = [bcast_step, norm_step]
                for tt in range(4):
                    for ec in range(2):
                        steps.append(y_step(tt, ec))
                return steps

            for b in range(B):
                for qc in range(NQC):
                    q0 = b * S + qc * QC
                    o_ps = [psop.tile([HD + 1, QC], F32,
                                      name=f"o{b}{qc}{h}", tag="o")
                            for h in range(HPC)]
                    p_tiles = []
                    for kt in range(NKT):
                        s_ps = psp.tile([128, HPC * QC], F32, tag="s")
                        for h in range(HPC):
                            nc.tensor.matmul(
                                s_ps[:, h * QC:(h + 1) * QC],
                                lhsT=kT_sb[h * HD:(h + 1) * HD,
                                           b * S + kt * KT:b * S + (kt + 1) * KT],
                                rhs=qT_sb[h * HD:(h + 1) * HD, q0:q0 + QC],
                                start=True, stop=True,
                            )
                        p_sb = pp.tile([128, HPC * QC], BF16, tag="p")
                        nc.scalar.activation(p_sb[:], s_ps[:], EXP,
                                             scale=float(EXPSCALE))
                        p_tiles.append(p_sb)
                        if pending:
                            pending.pop(0)()
                        if pending and len(pending) > NKT - 1 - kt:
                            pending.pop(0)()
                        if kt >= 6:
                            for h in range(HPC):
                                nc.tensor.matmul(
                                    o_ps[h][:],
                                    lhsT=vaug[b][:, kt - 6, h, :],
                                    rhs=p_tiles[kt - 6][:, h * QC:(h + 1) * QC],
                                    start=(kt - 6 == 0), stop=False,
                                )
                    for lag in range(NKT - 6, NKT):
                        for h in range(HPC):
                            nc.tensor.matmul(
                                o_ps[h][:],
                                lhsT=vaug[b][:, lag, h, :],
                                rhs=p_tiles[lag][:, h * QC:(h + 1) * QC],
                                start=False, stop=(lag == NKT - 1),
                            )
                    # free o_ps fast: raw o to SBUF bf16 + colsum rows
                    # (row 64) to a bf16 SBUF row for the broadcast matmul
                    oT_raw = oTp.tile([128, QC], BF16,
                                      name=f"oR{b}{qc}", tag="oTraw")
                    nc.vector.tensor_copy(oT_raw[0:64, :], o_ps[0][0:64, :])
                    nc.vector.tensor_copy(oT_raw[64:128, :], o_ps[1][0:64, :])
                    csrow = recipp.tile([128, 2 * QC], BF16,
                                        name=f"rc{b}{qc}", tag="recip")
                    nc.vector.tensor_copy(csrow[64:65, 0:512],
                                          o_ps[0][64:65, :])
                    nc.vector.tensor_copy(csrow[64:65, 512:1024],
                                          o_ps[1][64:65, :])
                    while pending:
                        pending.pop(0)()
                    epi = make_epilogue(b, qc, oT_raw, csrow)
                    # 18 > NKT: the 2-pops-per-kt drain rule clears the
                    # backlog within one chunk; ensures all of `extra` is
                    # emitted during b0 so b1 scores never lead their inputs.
                    take = min(len(extra), 18 - len(epi))
                    pending = extra[:take] + epi
                    del extra[:take]
            while pending:
                pending.pop(0)()
            if dbg:
                nc.sync.dma_start(dvaug[:], vaug[0][:])
                nc.sync.dma_start(dvt[:], vT_sb[:])
                nc.sync.dma_start(dqk[:], qkvT_sb[1][:])
    nc.compile()
    return nc


def _get_nc():
    if "nc" not in _CACHED:
        _CACHED["nc"] = _build_nc()
    return _CACHED["nc"]


def _host_prep(x, qkv_w, qkv_b, out_w):
    x = np.asarray(x, dtype=np.float32)
    qkv_w = np.asarray(qkv_w, dtype=np.float32)
    qkv_b = np.asarray(qkv_b, dtype=np.float32)
    out_w = np.asarray(out_w, dtype=np.float32)

    xTf = np.ascontiguousarray(x.reshape(T, H).T)
    xT = xTf.astype(BF16_NP)
    # fp8 x in DoubleRow layout: [NDR, 128, 2, T]
    xf8 = np.ascontiguousarray(
        xTf.reshape(NDR, 2, 128, T).transpose(0, 2, 1, 3)
    ).astype(FP8_NP)

    in_maps = []
    for c in range(NCORES):
        wq = qkv_w[128 * c:128 * c + 128]          # [128, H]
        wk = qkv_w[H + 128 * c:H + 128 * c + 128]
        wv_c = qkv_w[2 * H + 128 * c:2 * H + 128 * c + 128]
        # wqk8: [NDR, 128, 2, 256]; cols 0-127 = q feats, 128-255 = k feats
        wqk = np.concatenate([wq, wk], 0).T * WSCALE   # [H, 256]
        wqk8 = np.ascontiguousarray(
            wqk.reshape(NDR, 2, 128, 256).transpose(0, 2, 1, 3)
        ).astype(FP8_NP)
        # wv: [HKT, 128, DPC] = hidden-major slices of wv_c.T
        wvT = wv_c.T                                    # [H, DPC]
        wv8 = np.ascontiguousarray(
            wvT.reshape(HKT, 128, DPC)).astype(BF16_NP)
        woTa = np.ascontiguousarray(
            out_w[:, DPC * c:DPC * (c + 1)].T).astype(BF16_NP)  # [DPC, H]
        bqk = np.stack(
            [qkv_b[fg * H + 128 * c:fg * H + 128 * c + 128] for fg in range(2)],
            axis=1,
        ).astype(np.float32)
        vb = np.broadcast_to(
            qkv_b[2 * H + 128 * c:2 * H + 128 * c + 128][None, :], (128, DPC)
        ).astype(np.float32)
        in_maps.append({
            "xT": xT,
            "xf8": xf8,
            "wqk8": wqk8,
            "wv": wv8,
            "woT": woTa,
            "bqk": np.ascontiguousarray(bqk),
            "vbias": np.ascontiguousarray(vb),
        })
    return in_maps


def _run(in_maps, trace=False):
    if trace and "antenv.axon_hooks" not in sys.modules:
        try:
            import trn_agent_boot.trn_boot as _tb
            _hook = _tb._ntff_profile_via_ctypes("/opt/axon/libaxon_pjrt.so")
            _m = types.ModuleType("antenv.axon_hooks")
            _m.get_axon_ntff_profile_hook = lambda: _hook
            sys.modules["antenv.axon_hooks"] = _m
        except Exception:
            trace = False
    from concourse.bass_utils import run_bass_kernel_spmd

    nc = _get_nc()
    res = run_bass_kernel_spmd(nc, in_maps, core_ids=list(range(NCORES)), trace=trace)
    return res


def kernel(x, qkv_w, qkv_b, out_w, out_b):
    in_maps = _host_prep(x, qkv_w, qkv_b, out_w)
    res = _run(in_maps, trace=False)
    total = np.zeros((T, H), np.float32)
    for c in range(NCORES):
        total += res.results[c]["out"].astype(np.float32)
    total += np.asarray(out_b, dtype=np.float32)[None, :]
    return total.reshape(B, S, H)
